# revision 1
# baseline (speedup 1.0000x reference)
"""JKNet-Maxpool GNN kernel for 8 Trainium2 NeuronCores.

Strategy (graph/data parallel, dense-adjacency aggregation):
  - Shard dst nodes 8 ways (1250/core, padded to 1280 = 10 tiles of 128).
  - segment_sum over edges == A @ m with A[dst, src] the edge-count matrix.
    A entries are small ints -> exact in bf16.  Aggregation runs on the PE as
    dense matmuls: stationary = m chunks [128 src, 128 feat] (bf16), moving =
    A^T chunks [128 src, <=512 dst] (bf16, streamed from HBM), accumulated in
    fp32 PSUM over all 80 src tiles.
  - Transposed dataflow: activations live as x^T [feat_part, node_free], so
    the per-layer GEMM (fp32 for accuracy) uses x^T chunks as the stationary
    operand and W as the moving operand with zero transposes anywhere.
  - Per layer: local GEMM -> cast bf16 -> AllGather m across the 8 cores ->
    dense aggregation -> ReLU+bias -> running JK max.
  - Final GCN layer uses the normalization-weighted adjacency Aw (built on
    host, includes the self-loop 1/deg diagonal), then log_softmax.
"""

import numpy as np
import ml_dtypes

import concourse.bass as bass
import concourse.bacc as bacc
import concourse.mybir as mybir
import concourse.tile as tile
from concourse.bass_utils import run_bass_kernel_spmd
from concourse.masks import make_identity

BF16 = mybir.dt.bfloat16
F32 = mybir.dt.float32
AF = mybir.ActivationFunctionType
ALU = mybir.AluOpType
AX = mybir.AxisListType

# ---------------------------------------------------------------- config
class Cfg:
    def __init__(self, n_nodes, in_feats, units, out_feats, n_layers, n_cores=8):
        self.P = 128
        self.C = n_cores
        self.N = n_nodes
        self.IN = in_feats            # multiple of 128
        self.U = units                # multiple of 128
        self.OUTP = 128               # padded out feats (real out <= 128)
        self.L = n_layers             # hidden GCN layers
        nloc_real = (n_nodes + n_cores - 1) // n_cores
        self.NLOC_REAL = nloc_real
        self.NT_LOC = (nloc_real + 127) // 128
        self.NLOC = self.NT_LOC * 128            # padded local nodes
        self.KT = self.C * self.NT_LOC           # src tiles over padded space
        self.NFULL = self.KT * 128
        self.KT_IN = in_feats // 128
        self.KT_U = units // 128
        # moving-dim slices for aggregation matmuls (<=512 each)
        self.SLICES = []
        off = 0
        while off < self.NLOC:
            w = min(512, self.NLOC - off)
            self.SLICES.append((off, w))
            off += w


REAL = Cfg(n_nodes=10000, in_feats=512, units=256, out_feats=64, n_layers=6)
OUT_REAL = 64


# ---------------------------------------------------------------- program

DMA_ENGINE = "gpsimd"  # "sync" (HWDGE) or "gpsimd" (SWDGE)


def _dma(nc):
    return (nc.gpsimd if DMA_ENGINE == "gpsimd" else nc.sync).dma_start

def build_nc(cfg: Cfg) -> bass.Bass:
    nc = bacc.Bacc("TRN2", target_bir_lowering=False, num_devices=cfg.C)
    P, L = cfg.P, cfg.L

    # ---- dram I/O (per-core contents supplied via in_maps)
    hT_d = nc.dram_tensor("hT", [cfg.KT_IN, P, cfg.NLOC], F32, kind="ExternalInput")
    AT_d = nc.dram_tensor("AT", [cfg.KT, P, cfg.NLOC], BF16, kind="ExternalInput")
    AwT_d = nc.dram_tensor("AwT", [cfg.KT, P, cfg.NLOC], BF16, kind="ExternalInput")
    W0_d = nc.dram_tensor("W0", [cfg.KT_IN, P, cfg.U], F32, kind="ExternalInput")
    Wh_d = nc.dram_tensor("Wh", [L - 1, cfg.KT_U, P, cfg.U], F32, kind="ExternalInput")
    Wo_d = nc.dram_tensor("Wo", [cfg.KT_U, P, cfg.OUTP], F32, kind="ExternalInput")
    # packed per-partition biases: col l*2+ft = bias for layer l feat tile ft,
    # col 2L = final bias (bo padded)
    nb = 2 * L + 1 if cfg.KT_U == 2 else cfg.KT_U * L + 1
    nb = cfg.KT_U * L + 1
    bias_d = nc.dram_tensor("biases", [P, nb], F32, kind="ExternalInput")
    out_d = nc.dram_tensor("out", [cfg.NLOC, OUT_REAL], F32, kind="ExternalOutput")

    with tile.TileContext(nc) as tc:
        with (
            tc.tile_pool(name="const", bufs=1) as const_p,
            tc.tile_pool(name="wpool", bufs=1) as w_p,
            tc.tile_pool(name="xT", bufs=cfg.KT_IN + cfg.KT_U + 2) as x_p,
            tc.tile_pool(name="jk", bufs=1) as jk_p,
            tc.tile_pool(name="mfull", bufs=cfg.KT) as mf_p,
            tc.tile_pool(name="at", bufs=4) as at_p,
            tc.tile_pool(name="mloc", bufs=4) as ml_p,
            tc.tile_pool(name="small", bufs=6) as sm_p,
            tc.tile_pool(name="psmm", bufs=2, space="PSUM") as psmm_p,
            tc.tile_pool(name="psagg", bufs=2, space="PSUM") as psagg_p,
            tc.tile_pool(name="dram", bufs=1, space="DRAM") as dram_p,
        ):
            # ---- constants
            biases = const_p.tile([P, nb], F32, name="biases_sb")
            _dma(nc)(out=biases[:], in_=bias_d[:])
            ident = const_p.tile([P, P], F32, name="ident")
            make_identity(nc, ident[:])

            # ---- weights resident in SBUF
            w0_sb = []
            for k in range(cfg.KT_IN):
                t = w_p.tile([P, cfg.U], F32, name=f"w0_{k}")
                _dma(nc)(out=t[:], in_=W0_d[k])
                w0_sb.append(t)
            wh_sb = []
            for l in range(L - 1):
                row = []
                for k in range(cfg.KT_U):
                    t = w_p.tile([P, cfg.U], F32, name=f"wh_{l}_{k}")
                    _dma(nc)(out=t[:], in_=Wh_d[l, k])
                    row.append(t)
                wh_sb.append(row)
            wo_sb = []
            for k in range(cfg.KT_U):
                t = w_p.tile([P, cfg.OUTP], F32, name=f"wo_{k}")
                _dma(nc)(out=t[:], in_=Wo_d[k])
                wo_sb.append(t)

            # ---- x^T tiles (layer 0 = h^T)
            xT = []
            for k in range(cfg.KT_IN):
                t = x_p.tile([P, cfg.NLOC], F32, tag="xT", name=f"xt0_{k}")
                _dma(nc)(out=t[:], in_=hT_d[k])
                xT.append(t)

            # ---- JK running max tiles
            jk = [
                jk_p.tile([P, cfg.NLOC], F32, name=f"jk_{ft}")
                for ft in range(cfg.KT_U)
            ]

            # ---- collective bounce buffers
            m_loc_d = dram_p.tile([cfg.C, cfg.NT_LOC, P, cfg.U], BF16,
                                  name="m_loc_d")
            m_full_ds = [
                dram_p.tile([cfg.KT, P, cfg.U], BF16, name=f"m_full_d{l}",
                            addr_space="Shared")
                for l in range(L)
            ]
            mo_loc_d = dram_p.tile([cfg.C, cfg.NT_LOC, P, cfg.OUTP], BF16,
                                   name="mo_loc_d")
            mo_full_d = dram_p.tile([cfg.KT, P, cfg.OUTP], BF16,
                                    name="mo_full_d", addr_space="Shared")

            def gemm_allgather(xT_tiles, w_tiles, width, loc_d, full_d, lname):
                """m_loc = x_loc @ W (fp32), cast bf16, all-gather to SBUF tiles."""
                kt = len(xT_tiles)
                for nt in range(cfg.NT_LOC):
                    ps = psmm_p.tile([P, width], F32, tag="mm",
                                     name=f"ps_{lname}_{nt}")
                    for k in range(kt):
                        nc.tensor.matmul(
                            ps[:],
                            lhsT=xT_tiles[k][:, nt * P:(nt + 1) * P],
                            rhs=w_tiles[k][:],
                            start=(k == 0),
                            stop=(k == kt - 1),
                        )
                    mt = ml_p.tile([P, width], BF16, tag="mloc",
                                   name=f"m_{lname}_{nt}")
                    nc.vector.tensor_copy(out=mt[:], in_=ps[:])
                    # every core writes its shard into slot 0 of loc_d; the
                    # AllGather concatenates shards in replica order.
                    _dma(nc)(out=loc_d[0, nt], in_=mt[:])
                nc.gpsimd.collective_compute(
                    "AllGather",
                    ALU.bypass,
                    replica_groups=[list(range(cfg.C))],
                    ins=[loc_d[0].opt()],
                    outs=[full_d.opt()],
                )
                full_sb = []
                for k in range(cfg.KT):
                    t = mf_p.tile([P, width], BF16, tag="mfull",
                                  name=f"mf_{lname}_{k}")
                    _dma(nc)(out=t[:], in_=full_d[k])
                    full_sb.append(t)
                return full_sb

            def aggregate(full_sb, adjT_d, width, lname):
                """agg^T[feat, dst] += m_chunk.T @ A^T chunk, fp32 psum."""
                nft = width // P
                ps_list = [
                    psagg_p.tile([P, cfg.NLOC], F32, tag="agg",
                                 name=f"agg_{lname}_{ft}")
                    for ft in range(nft)
                ]
                for k in range(cfg.KT):
                    at = at_p.tile([P, cfg.NLOC], BF16, tag="at",
                                   name=f"at_{lname}_{k}")
                    _dma(nc)(out=at[:], in_=adjT_d[k])
                    for ft in range(nft):
                        for off, w in cfg.SLICES:
                            nc.tensor.matmul(
                                ps_list[ft][:, off:off + w],
                                lhsT=full_sb[k][:, ft * P:(ft + 1) * P],
                                rhs=at[:, off:off + w],
                                start=(k == 0),
                                stop=(k == cfg.KT - 1),
                            )
                return ps_list

            # ================= hidden layers =================
            for l in range(L):
                xt_in = xT
                w_tiles = w0_sb if l == 0 else wh_sb[l - 1]
                m_sb = gemm_allgather(xt_in, w_tiles, cfg.U,
                                      m_loc_d, m_full_ds[l], f"l{l}")
                ps_list = aggregate(m_sb, AT_d, cfg.U, f"l{l}")
                xT = []
                for ft in range(cfg.KT_U):
                    xt_new = x_p.tile([P, cfg.NLOC], F32, tag="xT",
                                      name=f"xt{l + 1}_{ft}")
                    nc.scalar.activation(
                        xt_new[:], ps_list[ft][:], AF.Relu,
                        bias=biases[:, cfg.KT_U * l + ft:cfg.KT_U * l + ft + 1],
                    )
                    xT.append(xt_new)
                    if l == 0:
                        nc.vector.tensor_copy(out=jk[ft][:], in_=xt_new[:])
                    else:
                        nc.vector.tensor_tensor(
                            out=jk[ft][:], in0=jk[ft][:], in1=xt_new[:],
                            op=ALU.max,
                        )

            # ================= final layer =================
            mo_sb = gemm_allgather(jk, wo_sb, cfg.OUTP,
                                   mo_loc_d, mo_full_d, "fin")
            ps_fin = aggregate(mo_sb, AwT_d, cfg.OUTP, "fin")[0]
            aggF = x_p.tile([P, cfg.NLOC], F32, tag="xT", name="aggF")
            nc.scalar.activation(
                aggF[:], ps_fin[:], AF.Identity,
                bias=biases[:, cfg.KT_U * L:cfg.KT_U * L + 1],
            )
            for nt in range(cfg.NT_LOC):
                ps_t = psmm_p.tile([P, P], F32, tag="mm", name=f"pst_{nt}")
                nc.tensor.transpose(
                    out=ps_t[:], in_=aggF[:, nt * P:(nt + 1) * P],
                    identity=ident[:],
                )
                z = ps_t[:, 0:OUT_REAL]
                rmax = sm_p.tile([P, 1], F32, tag="r1", name=f"rmax_{nt}")
                nc.vector.reduce_max(rmax[:], z, axis=AX.X)
                z2 = sm_p.tile([P, OUT_REAL], F32, tag="z2", name=f"z2_{nt}")
                nc.vector.tensor_scalar_sub(z2[:], z, rmax[:])
                ez = sm_p.tile([P, OUT_REAL], F32, tag="ez", name=f"ez_{nt}")
                nc.scalar.activation(ez[:], z2[:], AF.Exp)
                ssum = sm_p.tile([P, 1], F32, tag="r2", name=f"ssum_{nt}")
                nc.vector.reduce_sum(ssum[:], ez[:], axis=AX.X)
                lsum = sm_p.tile([P, 1], F32, tag="r3", name=f"lsum_{nt}")
                nc.scalar.activation(lsum[:], ssum[:], AF.Ln)
                o = sm_p.tile([P, OUT_REAL], F32, tag="o", name=f"o_{nt}")
                nc.vector.tensor_scalar_sub(o[:], z2[:], lsum[:])
                _dma(nc)(out=out_d[nt * P:(nt + 1) * P, :], in_=o[:])

    nc.compile()
    return nc


# ---------------------------------------------------------------- host prep
def host_prep(cfg: Cfg, h, edge_index, W0, b0, Wh, bh, Wo, bo):
    """Build per-core input maps."""
    bf = ml_dtypes.bfloat16
    N, C = cfg.N, cfg.C
    nlr, nloc = cfg.NLOC_REAL, cfg.NLOC
    src = np.asarray(edge_index[0], np.int64)
    dst = np.asarray(edge_index[1], np.int64)

    deg = np.zeros(N, np.float64)
    np.add.at(deg, dst, 1.0)
    deg += 1.0
    dinv = (deg ** -0.5).astype(np.float32)
    deg32 = deg.astype(np.float32)

    # padded global src index: core r, local i -> r*nloc + i
    def pad_idx(g):
        return (g // nlr) * nloc + (g % nlr)

    psrc = pad_idx(src)

    in_maps = []
    for c in range(C):
        lo, hi = c * nlr, min((c + 1) * nlr, N)
        nl = hi - lo
        sel = (dst >= lo) & (dst < hi)
        s_c = psrc[sel]
        d_c = (dst[sel] - lo).astype(np.int64)

        AT = np.zeros((cfg.NFULL, nloc), np.float32)
        np.add.at(AT, (s_c, d_c), 1.0)

        cw = dinv[src[sel]] * dinv[dst[sel]]
        AwT = np.zeros((cfg.NFULL, nloc), np.float32)
        np.add.at(AwT, (s_c, d_c), cw.astype(np.float64).astype(np.float32))
        # self loop 1/deg on the (padded) diagonal
        gids = np.arange(lo, hi)
        AwT[pad_idx(gids), gids - lo] += 1.0 / deg32[gids]

        hT = np.zeros((cfg.IN, nloc), np.float32)
        hT[:, :nl] = np.asarray(h[lo:hi], np.float32).T

        nb = cfg.KT_U * cfg.L + 1
        biases = np.zeros((128, nb), np.float32)
        for l in range(cfg.L):
            b = np.asarray(b0 if l == 0 else bh[l - 1], np.float32)
            for ft in range(cfg.KT_U):
                biases[:, cfg.KT_U * l + ft] = b[ft * 128:(ft + 1) * 128]
        bo_arr = np.asarray(bo, np.float32)
        biases[:len(bo_arr), cfg.KT_U * cfg.L] = bo_arr

        Wo_pad = np.zeros((cfg.U, cfg.OUTP), np.float32)
        Wo_pad[:, :np.asarray(Wo).shape[1]] = np.asarray(Wo, np.float32)

        in_maps.append({
            "hT": hT.reshape(cfg.KT_IN, 128, nloc).copy(),
            "AT": AT.astype(bf).reshape(cfg.KT, 128, nloc).copy(),
            "AwT": AwT.astype(bf).reshape(cfg.KT, 128, nloc).copy(),
            "W0": np.asarray(W0, np.float32).reshape(cfg.KT_IN, 128, cfg.U).copy(),
            "Wh": np.asarray(Wh, np.float32).reshape(cfg.L - 1, cfg.KT_U, 128, cfg.U).copy(),
            "Wo": Wo_pad.reshape(cfg.KT_U, 128, cfg.OUTP).copy(),
            "biases": biases,
        })
    return in_maps


_CACHE = {}


def _get_nc():
    if "nc" not in _CACHE:
        _CACHE["nc"] = build_nc(REAL)
    return _CACHE["nc"]


def kernel(h, edge_index, W0, b0, Wh, bh, Wo, bo, _trace=False, _trace_kwargs=None):
    cfg = REAL
    nc = _get_nc()
    in_maps = host_prep(cfg, h, edge_index, W0, b0, Wh, bh, Wo, bo)
    res = run_bass_kernel_spmd(
        nc, in_maps, list(range(cfg.C)),
        trace=_trace, **(_trace_kwargs or {}),
    )
    outs = [np.asarray(res.results[c]["out"])[:cfg.NLOC_REAL] for c in range(cfg.C)]
    full = np.concatenate(outs, axis=0)[:cfg.N].astype(np.float32)
    if _trace:
        return full, res
    return full



# revision 7
# speedup vs baseline: 1.6798x; 1.6798x over previous
"""JKNet-Maxpool GNN kernel for 8 Trainium2 NeuronCores.

Strategy (graph/data parallel, dense-adjacency aggregation):
  - Shard dst nodes 8 ways (1250/core, padded to 1280 = 10 tiles of 128).
  - segment_sum over edges == A @ m with A[dst, src] the edge-count matrix.
    A^T is stored fp8 (e4m3; counts are small ints -> exact) and stays
    RESIDENT in SBUF for all 7 layers (~100KB/partition) - no per-layer
    HBM streaming of the adjacency.
  - Per layer: local GEMM (bf16) -> scale by beta_l, cast to fp8 ->
    2 chunked AllGathers (feature halves) -> dense aggregation on the PE
    with fp8 DoubleRow matmuls (2 src-tiles contracted per instruction) ->
    ReLU(agg/beta_l + b) -> running JK max.  The ft=0 aggregation overlaps
    the ft=1 AllGather.
  - beta_l scales keep fp8 operands in range; they are computed on host
    from a cheap fp32 forward pass (values grow ~16x/layer: Perron growth
    of the un-normalized GCN) and enter the device as a runtime tensor.
  - Final GCN layer (normalize=True) reuses the SAME resident A^T:
    coef = dinv[src]*dinv[dst] is separable, so m is pre-scaled by
    dinv[src] before the gather and agg^T is post-scaled by dinv[dst]
    after transpose; the self-loop term m/deg is added locally.
  - log_softmax per node tile, then DMA out.
"""

import os
import numpy as np
import ml_dtypes

import concourse.bass as bass
import concourse.bacc as bacc
import concourse.mybir as mybir
import concourse.tile as tile
from concourse.bass_utils import run_bass_kernel_spmd
from concourse.masks import make_identity

BF16 = mybir.dt.bfloat16
F32 = mybir.dt.float32
F8 = mybir.dt.float8e4
AF = mybir.ActivationFunctionType
ALU = mybir.AluOpType
AX = mybir.AxisListType
DR = mybir.MatmulPerfMode.DoubleRow

MODE = os.environ.get("KMODE", "fp8")  # "bf16" | "fp8" | "fp8hl"

P = 128
C = 8
N_NODES = 10000
IN_FEATS = 512
UNITS = 256
OUT_REAL = 64
OUTP = 128
L = 6
NLOC_REAL = 1250
NT = 10           # local node tiles
NLOC = NT * P     # 1280
KT = C * NT       # 80 src tiles
NPAIR = KT // 2   # 40 src pair tiles
KT_IN = IN_FEATS // P   # 4
KT_U = UNITS // P       # 2
NSC = 16          # scales vector width


def build_nc(mode: str) -> bass.Bass:
    nc = bacc.Bacc("TRN2", target_bir_lowering=False, num_devices=C)
    fp8 = mode != "bf16"
    mdt = F8 if fp8 else BF16
    parts = 2 if mode == "fp8hl" else 1
    # aggregation moving-dim slices over the 1280 local dst columns
    if fp8:
        slices = [(o, 256) for o in range(0, NLOC, 256)]
    else:
        slices = [(0, 512), (512, 512), (1024, 256)]

    hT_d = nc.dram_tensor("hT", [KT_IN, P, NLOC], BF16, kind="ExternalInput")
    ATp_d = nc.dram_tensor("ATp", [NPAIR, P, 2, NLOC], F8, kind="ExternalInput")
    W0_d = nc.dram_tensor("W0", [KT_IN, P, UNITS], BF16, kind="ExternalInput")
    Wh_d = nc.dram_tensor("Wh", [L - 1, KT_U, P, UNITS], BF16, kind="ExternalInput")
    Wo_d = nc.dram_tensor("Wo", [KT_U, P, OUTP], BF16, kind="ExternalInput")
    bias_d = nc.dram_tensor("biases", [P, KT_U * L], F32, kind="ExternalInput")
    scal_d = nc.dram_tensor("scales", [P, NSC], F32, kind="ExternalInput")
    dsc_d = nc.dram_tensor("dscale", [P, NT], F32, kind="ExternalInput")
    ddst_d = nc.dram_tensor("dinv_dst", [P, NT], F32, kind="ExternalInput")
    idg_d = nc.dram_tensor("invdeg", [P, NT], F32, kind="ExternalInput")
    bo_d = nc.dram_tensor("bo_bc", [P, OUTP], F32, kind="ExternalInput")
    out_d = nc.dram_tensor("out", [NLOC, OUT_REAL], F32, kind="ExternalOutput")
    dump = os.environ.get("KDUMP") == "1"
    if dump:
        xd_d = nc.dram_tensor("xdump", [L, KT_U, P, NLOC], BF16,
                              kind="ExternalOutput")
        md_d = nc.dram_tensor("mdump", [L, P, NT, UNITS], F32,
                              kind="ExternalOutput")

    with tile.TileContext(nc) as tc:
        with (
            tc.tile_pool(name="const", bufs=1) as const_p,
            tc.tile_pool(name="atp", bufs=1) as at_p,
            tc.tile_pool(name="wp", bufs=1) as w_p,
            tc.tile_pool(name="xp", bufs=KT_IN + 2 * KT_U) as x_p,
            tc.tile_pool(name="jkp", bufs=1) as jk_p,
            tc.tile_pool(name="mlp", bufs=2) as ml_p,
            tc.tile_pool(name="mrp", bufs=(88 if mode != "bf16" else 16)) as mr_p,
            tc.tile_pool(name="afp", bufs=1) as af_p,
            tc.tile_pool(name="zp", bufs=4) as z_p,
            tc.tile_pool(name="psmm", bufs=2, space="PSUM") as psmm_p,
            tc.tile_pool(name="psagg", bufs=2, space="PSUM") as psagg_p,
            tc.tile_pool(name="dram", bufs=1, space="DRAM") as dram_p,
        ):
            dma = nc.sync.dma_start

            # ---- small constants
            biases = const_p.tile([P, KT_U * L], F32, tag="biases_sb", name="biases_sb")
            dma(out=biases[:], in_=bias_d[:])
            scales = const_p.tile([P, NSC], F32, tag="scales_sb", name="scales_sb")
            dma(out=scales[:], in_=scal_d[:])
            dscale = const_p.tile([P, NT], F32, tag="dscale_sb", name="dscale_sb")
            dma(out=dscale[:], in_=dsc_d[:])
            dinv_dst = const_p.tile([P, NT], F32, tag="ddst_sb", name="ddst_sb")
            dma(out=dinv_dst[:], in_=ddst_d[:])
            invdeg = const_p.tile([P, NT], F32, tag="idg_sb", name="idg_sb")
            dma(out=invdeg[:], in_=idg_d[:])
            bo_bc = const_p.tile([P, OUTP], F32, tag="bo_sb", name="bo_sb")
            dma(out=bo_bc[:], in_=bo_d[:])
            ident = const_p.tile([P, P], F32, tag="ident", name="ident")
            make_identity(nc, ident[:])

            # ---- weights resident (bf16)
            w0_sb = []
            for k in range(KT_IN):
                t = w_p.tile([P, UNITS], BF16, tag=f"w0_{k}", name=f"w0_{k}")
                dma(out=t[:], in_=W0_d[k])
                w0_sb.append(t)
            wh_sb = []
            for l in range(L - 1):
                row = []
                for k in range(KT_U):
                    t = w_p.tile([P, UNITS], BF16, tag=f"wh_{l}_{k}", name=f"wh_{l}_{k}")
                    dma(out=t[:], in_=Wh_d[l, k])
                    row.append(t)
                wh_sb.append(row)
            wo_sb = []
            for k in range(KT_U):
                t = w_p.tile([P, OUTP], BF16, tag=f"wo_{k}", name=f"wo_{k}")
                dma(out=t[:], in_=Wo_d[k])
                wo_sb.append(t)

            # ---- A^T resident in SBUF, fp8, pair layout [128, 2, 1280]
            at_sb = []
            for p in range(NPAIR):
                t = at_p.tile([P, 2, NLOC], F8, tag=f"at{p}", name=f"at_{p}")
                dma(out=t[:], in_=ATp_d[p])
                at_sb.append(t)

            # ---- x^T tiles (layer 0 = h^T), bf16
            xT = []
            for k in range(KT_IN):
                t = x_p.tile([P, NLOC], BF16, tag="xT", name=f"xt0_{k}")
                dma(out=t[:], in_=hT_d[k])
                xT.append(t)

            jk = [jk_p.tile([P, NLOC], BF16, tag=f"jk{ft}", name=f"jk_{ft}")
                  for ft in range(KT_U)]

            # ---- collective bounce buffers (per layer, per feat chunk, per part)
            loc_h = [[[dram_p.tile([NT // 2, P, 2, P], mdt,
                                   tag=f"loc{l}_{ch}_{pt}", name=f"loc{l}_{ch}_{pt}")
                       for pt in range(parts)] for ch in range(KT_U)]
                     for l in range(L)]
            full_h = [[[dram_p.tile([NPAIR, P, 2, P], mdt,
                                    tag=f"full{l}_{ch}_{pt}", name=f"full{l}_{ch}_{pt}",
                                    addr_space="Shared")
                        for pt in range(parts)] for ch in range(KT_U)]
                      for l in range(L)]
            loc_f = [dram_p.tile([NT // 2, P, 2, OUTP], mdt, tag=f"locF_{pt}", name=f"locF_{pt}")
                     for pt in range(parts)]
            full_f = [dram_p.tile([NPAIR, P, 2, OUTP], mdt, tag=f"fullF_{pt}", name=f"fullF_{pt}",
                                  addr_space="Shared")
                      for pt in range(parts)]

            def cc(ins_t, outs_t):
                nc.gpsimd.collective_compute(
                    "AllGather", ALU.bypass,
                    replica_groups=[list(range(C))],
                    ins=[ins_t.opt()], outs=[outs_t.opt()],
                )

            def aggregate(ps, full_list, width, lname):
                """DMA all gathered pairs, then aggregate into psum ps.

                fp8/DR path runs slice-major: each psum accumulation group
                (column range) executes contiguously -- interleaving DR
                matmuls across psum regions corrupts accumulation on HW.
                """
                mps = []
                for p in range(NPAIR):
                    row = []
                    for pt in range(parts):
                        mp = mr_p.tile([P, 2, width], mdt, tag="mr",
                                       name=f"mr_{lname}_{p}_{pt}")
                        dma(out=mp[:], in_=full_list[pt][p])
                        row.append(mp)
                    mps.append(row)
                if fp8:
                    for off, w in slices:
                        for p in range(NPAIR):
                            for ipt in range(parts):
                                nc.tensor.matmul(
                                    ps[:, off:off + w],
                                    lhsT=mps[p][ipt][:],
                                    rhs=at_sb[p][:, :, off:off + w],
                                    start=(p == 0 and ipt == 0),
                                    stop=(p == NPAIR - 1 and ipt == parts - 1),
                                    perf_mode=DR,
                                )
                else:
                    for p in range(NPAIR):
                        for i in range(2):
                            for off, w in slices:
                                nc.tensor.matmul(
                                    ps[:, off:off + w],
                                    lhsT=mps[p][0][:, i, :],
                                    rhs=at_sb[p][:, i, off:off + w],
                                    start=(p == 0 and i == 0),
                                    stop=(p == NPAIR - 1 and i == 1),
                                )

            # ================= hidden layers =================
            for l in range(L):
                w_tiles = w0_sb if l == 0 else wh_sb[l - 1]
                kt = len(xT)
                mloc = ml_p.tile([P, NT, UNITS], mdt, tag="ml",
                                 name=f"ml_{l}")
                mlo = (ml_p.tile([P, NT, UNITS], mdt, tag="mlo",
                                 name=f"mlo_{l}") if parts == 2 else None)
                sc_b = scales[:, 2 * l:2 * l + 1]
                for nt in range(NT):
                    ps = psmm_p.tile([P, UNITS], F32, tag="mm",
                                     name=f"ps_{l}_{nt}")
                    for k in range(kt):
                        nc.tensor.matmul(
                            ps[:], lhsT=xT[k][:, nt * P:(nt + 1) * P],
                            rhs=w_tiles[k][:],
                            start=(k == 0), stop=(k == kt - 1),
                        )
                    # m_hi = fp8(ps * beta_l)
                    nc.scalar.activation(mloc[:, nt, :], ps[:], AF.Identity,
                                         scale=sc_b)
                    if dump:
                        md_t = z_p.tile([P, UNITS], F32, tag="mdmp",
                                        name=f"md_{l}_{nt}")
                        nc.vector.tensor_copy(out=md_t[:], in_=ps[:])
                        dma(out=md_d[l, :, nt, :], in_=md_t[:])
                    if parts == 2:
                        t1 = z_p.tile([P, UNITS], F32, tag="hl1",
                                      name=f"hl1_{l}_{nt}")
                        nc.vector.tensor_scalar(
                            out=t1[:], in0=ps[:], scalar1=sc_b, scalar2=None,
                            op0=ALU.mult)
                        t2 = z_p.tile([P, UNITS], F32, tag="hl2",
                                      name=f"hl2_{l}_{nt}")
                        nc.vector.tensor_copy(out=t2[:], in_=mloc[:, nt, :])
                        nc.vector.tensor_tensor(
                            out=t1[:], in0=t1[:], in1=t2[:], op=ALU.subtract)
                        nc.vector.tensor_copy(out=mlo[:, nt, :], in_=t1[:])
                for q in range(NT // 2):
                    for ch in range(KT_U):
                        dma(out=loc_h[l][ch][0][q],
                            in_=mloc[:, 2 * q:2 * q + 2, ch * P:(ch + 1) * P])
                        if parts == 2:
                            dma(out=loc_h[l][ch][1][q],
                                in_=mlo[:, 2 * q:2 * q + 2, ch * P:(ch + 1) * P])
                for ch in range(KT_U):
                    for pt in range(parts):
                        cc(loc_h[l][ch][pt], full_h[l][ch][pt])

                ps_agg = [psagg_p.tile([P, NLOC], F32, tag="agg",
                                       name=f"agg_{l}_{ft}")
                          for ft in range(KT_U)]
                for ft in range(KT_U):
                    aggregate(ps_agg[ft], full_h[l][ft], P, f"h{l}_{ft}")

                sc_u = scales[:, 2 * l + 1:2 * l + 2]
                xT = []
                for ft in range(KT_U):
                    xt_new = x_p.tile([P, NLOC], BF16, tag="xT",
                                      name=f"xt{l + 1}_{ft}")
                    nc.scalar.activation(
                        xt_new[:], ps_agg[ft][:], AF.Relu,
                        bias=biases[:, KT_U * l + ft:KT_U * l + ft + 1],
                        scale=sc_u,
                    )
                    xT.append(xt_new)
                    if dump:
                        dma(out=xd_d[l, ft], in_=xt_new[:])
                    if l == 0:
                        nc.vector.tensor_copy(out=jk[ft][:], in_=xt_new[:])
                    else:
                        nc.vector.tensor_tensor(
                            out=jk[ft][:], in0=jk[ft][:], in1=xt_new[:],
                            op=ALU.max)

            # ================= final layer =================
            mo = const_p.tile([P, NT, OUTP], F32, tag="mo_keep", name="mo_keep")
            mloc = ml_p.tile([P, NT, OUTP], mdt, tag="mlf", name="ml_fin")
            mlo = (ml_p.tile([P, NT, OUTP], mdt, tag="mlof", name="mlo_fin")
                   if parts == 2 else None)
            for nt in range(NT):
                psw = psmm_p.tile([P, UNITS], F32, tag="mm", name=f"ps_f_{nt}")
                ps = psw[:, 0:OUTP]
                for k in range(KT_U):
                    nc.tensor.matmul(
                        ps, lhsT=jk[k][:, nt * P:(nt + 1) * P],
                        rhs=wo_sb[k][:],
                        start=(k == 0), stop=(k == KT_U - 1),
                    )
                nc.scalar.activation(mo[:, nt, :], ps, AF.Identity)
                # m_s = fp8(ps * dinv * beta_f)
                nc.vector.tensor_scalar(
                    out=mloc[:, nt, :], in0=ps,
                    scalar1=dscale[:, nt:nt + 1], scalar2=None, op0=ALU.mult)
                if parts == 2:
                    t1 = z_p.tile([P, OUTP], F32, tag="hl1", name=f"hf1_{nt}")
                    nc.vector.tensor_scalar(
                        out=t1[:], in0=ps, scalar1=dscale[:, nt:nt + 1],
                        scalar2=None, op0=ALU.mult)
                    t2 = z_p.tile([P, OUTP], F32, tag="hl2", name=f"hf2_{nt}")
                    nc.vector.tensor_copy(out=t2[:], in_=mloc[:, nt, :])
                    nc.vector.tensor_tensor(out=t1[:], in0=t1[:], in1=t2[:],
                                            op=ALU.subtract)
                    nc.vector.tensor_copy(out=mlo[:, nt, :], in_=t1[:])
            for q in range(NT // 2):
                dma(out=loc_f[0][q], in_=mloc[:, 2 * q:2 * q + 2, :])
                if parts == 2:
                    dma(out=loc_f[1][q], in_=mlo[:, 2 * q:2 * q + 2, :])
            for pt in range(parts):
                cc(loc_f[pt], full_f[pt])

            ps_fin = psagg_p.tile([P, NLOC], F32, tag="agg", name="agg_fin")
            aggregate(ps_fin, full_f, OUTP, "fin")

            aggF = af_p.tile([P, NLOC], F32, tag="aggF", name="aggF")
            nc.scalar.activation(aggF[:], ps_fin[:], AF.Identity,
                                 scale=scales[:, 13:14])
            for nt in range(NT):
                ps_tw = psmm_p.tile([P, UNITS], F32, tag="mm", name=f"pst_{nt}")
                ps_t = ps_tw[:, 0:P]
                nc.tensor.transpose(
                    out=ps_t, in_=aggF[:, nt * P:(nt + 1) * P],
                    identity=ident[:],
                )
                z = z_p.tile([P, OUT_REAL], F32, tag="z", name=f"z_{nt}")
                nc.vector.tensor_scalar(
                    out=z[:], in0=ps_tw[:, 0:OUT_REAL],
                    scalar1=dinv_dst[:, nt:nt + 1], scalar2=None, op0=ALU.mult)
                t2 = z_p.tile([P, OUT_REAL], F32, tag="zb", name=f"zb_{nt}")
                nc.vector.tensor_scalar(
                    out=t2[:], in0=mo[:, nt, 0:OUT_REAL],
                    scalar1=invdeg[:, nt:nt + 1], scalar2=None, op0=ALU.mult)
                nc.vector.tensor_tensor(out=z[:], in0=z[:], in1=t2[:],
                                        op=ALU.add)
                nc.vector.tensor_tensor(out=z[:], in0=z[:],
                                        in1=bo_bc[:, 0:OUT_REAL], op=ALU.add)
                # log_softmax over the 64 real feats
                rmax = z_p.tile([P, 1], F32, tag="r1", name=f"rmax_{nt}")
                nc.vector.reduce_max(rmax[:], z[:], axis=AX.X)
                nc.vector.tensor_scalar_sub(z[:], z[:], rmax[:])
                ez = z_p.tile([P, OUT_REAL], F32, tag="ez", name=f"ez_{nt}")
                nc.scalar.activation(ez[:], z[:], AF.Exp)
                ssum = z_p.tile([P, 1], F32, tag="r2", name=f"ssum_{nt}")
                nc.vector.reduce_sum(ssum[:], ez[:], axis=AX.X)
                lsum = z_p.tile([P, 1], F32, tag="r3", name=f"lsum_{nt}")
                nc.scalar.activation(lsum[:], ssum[:], AF.Ln)
                o = z_p.tile([P, OUT_REAL], F32, tag="o", name=f"o_{nt}")
                nc.vector.tensor_scalar_sub(o[:], z[:], lsum[:])
                dma(out=out_d[nt * P:(nt + 1) * P, :], in_=o[:])

    nc.compile()
    return nc


# ---------------------------------------------------------------- host prep
def host_prep(h, edge_index, W0, b0, Wh, bh, Wo, bo, mode):
    bf = ml_dtypes.bfloat16
    f8 = ml_dtypes.float8_e4m3
    N = N_NODES
    nlr, nloc = NLOC_REAL, NLOC
    src = np.asarray(edge_index[0], np.int64)
    dst = np.asarray(edge_index[1], np.int64)
    h32 = np.asarray(h, np.float32)
    W0_32 = np.asarray(W0, np.float32)
    Wh_32 = np.asarray(Wh, np.float32)
    Wo_32 = np.asarray(Wo, np.float32)
    b0_32 = np.asarray(b0, np.float32)
    bh_32 = np.asarray(bh, np.float32)
    bo_32 = np.asarray(bo, np.float32)

    deg = np.zeros(N, np.float64)
    np.add.at(deg, dst, 1.0)
    degf = (deg + 1.0).astype(np.float32)
    dinv = degf ** -0.5

    # ---- cheap fp32 forward pass for the fp8 beta_l scales (sort+reduceat
    # segment sum; ~1s).  Only the 7 max|m| scalars are used.
    if mode != "bf16":
        order = np.argsort(dst, kind="stable")
        ds = dst[order]
        ss = src[order]
        seg_starts = np.flatnonzero(np.r_[True, ds[1:] != ds[:-1]])
        seg_ids = ds[seg_starts]

        def seg_sum(m):
            gath = m[ss]
            red = np.add.reduceat(gath, seg_starts, axis=0)
            out = np.zeros((N, m.shape[1]), np.float32)
            out[seg_ids] = red
            return out

        x = h32
        mmax = []
        for l in range(L):
            W = W0_32 if l == 0 else Wh_32[l - 1]
            b = b0_32 if l == 0 else bh_32[l - 1]
            m = x @ W
            mmax.append(float(np.abs(m).max()))
            x = np.maximum(seg_sum(m) + b, 0)
            if l == 0:
                xj = x.copy()
            else:
                np.maximum(xj, x, out=xj)
        mo = xj @ Wo_32
        mfmax = float(np.abs(mo * dinv[:, None]).max())
        betas = [100.0 / max(v, 1e-30) for v in mmax]
        betaf = 100.0 / max(mfmax, 1e-30)
    else:
        betas = [1.0] * L
        betaf = 1.0

    def pad_idx(g):
        return (g // nlr) * nloc + (g % nlr)

    psrc = pad_idx(src)

    scales = np.zeros((P, NSC), np.float32)
    for l in range(L):
        scales[:, 2 * l] = betas[l]
        scales[:, 2 * l + 1] = 1.0 / betas[l]
    scales[:, 13] = 1.0 / betaf

    biases = np.zeros((P, KT_U * L), np.float32)
    for l in range(L):
        b = b0_32 if l == 0 else bh_32[l - 1]
        for ft in range(KT_U):
            biases[:, KT_U * l + ft] = b[ft * P:(ft + 1) * P]
    bo_bc = np.zeros((P, OUTP), np.float32)
    bo_bc[:, :OUT_REAL] = bo_32[None, :]

    Wo_pad = np.zeros((UNITS, OUTP), np.float32)
    Wo_pad[:, :OUT_REAL] = Wo_32

    NFULL = KT * P
    in_maps = []
    for c in range(C):
        lo, hi = c * nlr, min((c + 1) * nlr, N)
        nl = hi - lo
        sel = (dst >= lo) & (dst < hi)
        s_c = psrc[sel]
        d_c = (dst[sel] - lo).astype(np.int64)

        AT = np.zeros((NFULL, nloc), np.float32)
        np.add.at(AT, (s_c, d_c), 1.0)
        ATp = AT.reshape(NPAIR, 2, P, nloc).transpose(0, 2, 1, 3)

        hT = np.zeros((IN_FEATS, nloc), np.float32)
        hT[:, :nl] = h32[lo:hi].T

        gl = np.arange(lo, hi)
        dloc = np.ones(nloc, np.float32)
        dloc[:nl] = dinv[gl]
        dsc = (dloc * betaf).reshape(NT, P).T.copy()
        ddst = dloc.reshape(NT, P).T.copy()
        idg = np.ones(nloc, np.float32)
        idg[:nl] = 1.0 / degf[gl]
        idg = idg.reshape(NT, P).T.copy()

        in_maps.append({
            "hT": hT.reshape(KT_IN, P, nloc).astype(bf).copy(),
            "ATp": ATp.astype(f8).copy(),
            "W0": W0_32.reshape(KT_IN, P, UNITS).astype(bf).copy(),
            "Wh": Wh_32.reshape(L - 1, KT_U, P, UNITS).astype(bf).copy(),
            "Wo": Wo_pad.reshape(KT_U, P, OUTP).astype(bf).copy(),
            "biases": biases,
            "scales": scales,
            "dscale": dsc,
            "dinv_dst": ddst,
            "invdeg": idg,
            "bo_bc": bo_bc,
        })
    return in_maps


_CACHE = {}


def _get_nc():
    if "nc" not in _CACHE:
        _CACHE["nc"] = build_nc(MODE)
    return _CACHE["nc"]


def kernel(h, edge_index, W0, b0, Wh, bh, Wo, bo, _trace=False, _trace_kwargs=None):
    nc = _get_nc()
    in_maps = host_prep(h, edge_index, W0, b0, Wh, bh, Wo, bo, MODE)
    res = run_bass_kernel_spmd(
        nc, in_maps, list(range(C)),
        trace=_trace, **(_trace_kwargs or {}),
    )
    outs = [np.asarray(res.results[c]["out"])[:NLOC_REAL] for c in range(C)]
    full = np.concatenate(outs, axis=0)[:N_NODES].astype(np.float32)
    if _trace:
        return full, res
    return full


# revision 8
# speedup vs baseline: 2.2973x; 1.3676x over previous
"""JKNet-Maxpool GNN kernel for 8 Trainium2 NeuronCores.

Strategy (graph/data parallel, dense-adjacency aggregation):
  - Shard dst nodes 8 ways (1250/core, padded to 1280 = 10 tiles of 128).
  - segment_sum over edges == A @ m with A[dst, src] the edge-count matrix.
    A^T is stored fp8 (e4m3; counts are small ints -> exact) and stays
    RESIDENT in SBUF for all 7 layers (~100KB/partition) - no per-layer
    HBM streaming of the adjacency.
  - Per layer: local GEMM (bf16) -> scale by beta_l, cast to fp8 ->
    2 chunked AllGathers (feature halves) -> dense aggregation on the PE
    with fp8 DoubleRow matmuls (2 src-tiles contracted per instruction) ->
    ReLU(agg/beta_l + b) -> running JK max.  The ft=0 aggregation overlaps
    the ft=1 AllGather.
  - beta_l scales keep fp8 operands in range; they are computed on host
    from a cheap fp32 forward pass (values grow ~16x/layer: Perron growth
    of the un-normalized GCN) and enter the device as a runtime tensor.
  - Final GCN layer (normalize=True) reuses the SAME resident A^T:
    coef = dinv[src]*dinv[dst] is separable, so m is pre-scaled by
    dinv[src] before the gather and agg^T is post-scaled by dinv[dst]
    after transpose; the self-loop term m/deg is added locally.
  - log_softmax per node tile, then DMA out.
"""

import os
import numpy as np
import ml_dtypes

import concourse.bass as bass
import concourse.bacc as bacc
import concourse.mybir as mybir
import concourse.tile as tile
from concourse.bass_utils import run_bass_kernel_spmd
from concourse.masks import make_identity

BF16 = mybir.dt.bfloat16
F32 = mybir.dt.float32
F8 = mybir.dt.float8e4
AF = mybir.ActivationFunctionType
ALU = mybir.AluOpType
AX = mybir.AxisListType
DR = mybir.MatmulPerfMode.DoubleRow

MODE = os.environ.get("KMODE", "fp8")  # "bf16" | "fp8" | "fp8hl"

P = 128
C = 8
N_NODES = 10000
IN_FEATS = 512
UNITS = 256
OUT_REAL = 64
OUTP = 128
L = 6
NLOC_REAL = 1250
NT = 10           # local node tiles
NLOC = NT * P     # 1280
KT = C * NT       # 80 src tiles
NPAIR = KT // 2   # 40 src pair tiles
KT_IN = IN_FEATS // P   # 4
KT_U = UNITS // P       # 2
NSC = 16          # scales vector width


def build_nc(mode: str) -> bass.Bass:
    nc = bacc.Bacc("TRN2", target_bir_lowering=False, num_devices=C)
    fp8 = mode != "bf16"
    mdt = F8 if fp8 else BF16
    parts = 2 if mode == "fp8hl" else 1
    # aggregation moving-dim slices over the 1280 local dst columns;
    # each slice accumulates in its own (bank-aligned) psum tile: DR
    # matmuls interleaving accumulation groups within one psum tile
    # corrupt results on HW.
    SLICE_W = 256 if fp8 else 512
    slices = []
    off = 0
    while off < NLOC:
        slices.append((off, min(SLICE_W, NLOC - off)))
        off += SLICE_W

    hT_d = nc.dram_tensor("hT", [KT_IN, P, NLOC], BF16, kind="ExternalInput")
    ATp_d = nc.dram_tensor("ATp", [NPAIR, P, 2, NLOC], F8, kind="ExternalInput")
    W0_d = nc.dram_tensor("W0", [KT_IN, P, UNITS], BF16, kind="ExternalInput")
    Wh_d = nc.dram_tensor("Wh", [L - 1, KT_U, P, UNITS], BF16, kind="ExternalInput")
    Wo_d = nc.dram_tensor("Wo", [KT_U, P, OUTP], BF16, kind="ExternalInput")
    bias_d = nc.dram_tensor("biases", [P, KT_U * L], F32, kind="ExternalInput")
    scal_d = nc.dram_tensor("scales", [P, NSC], F32, kind="ExternalInput")
    dsc_d = nc.dram_tensor("dscale", [P, NT], F32, kind="ExternalInput")
    ddst_d = nc.dram_tensor("dinv_dst", [P, NT], F32, kind="ExternalInput")
    idg_d = nc.dram_tensor("invdeg", [P, NT], F32, kind="ExternalInput")
    bo_d = nc.dram_tensor("bo_bc", [P, OUTP], F32, kind="ExternalInput")
    out_d = nc.dram_tensor("out", [NLOC, OUT_REAL], F32, kind="ExternalOutput")
    dump = os.environ.get("KDUMP") == "1"
    if dump:
        xd_d = nc.dram_tensor("xdump", [L, KT_U, P, NLOC], BF16,
                              kind="ExternalOutput")
        md_d = nc.dram_tensor("mdump", [L, P, NT, UNITS], F32,
                              kind="ExternalOutput")

    with tile.TileContext(nc) as tc:
        with (
            tc.tile_pool(name="const", bufs=1) as const_p,
            tc.tile_pool(name="atp", bufs=1) as at_p,
            tc.tile_pool(name="wp", bufs=1) as w_p,
            tc.tile_pool(name="xp", bufs=KT_IN + 2 * KT_U) as x_p,
            tc.tile_pool(name="jkp", bufs=1) as jk_p,
            tc.tile_pool(name="mlp", bufs=2) as ml_p,
            tc.tile_pool(name="mrp", bufs=24) as mr_p,
            tc.tile_pool(name="afp", bufs=1) as af_p,
            tc.tile_pool(name="zp", bufs=4) as z_p,
            tc.tile_pool(name="psmm", bufs=2, space="PSUM") as psmm_p,
            tc.tile_pool(name="psagg", bufs=1, space="PSUM") as psagg_p,
            tc.tile_pool(name="dram", bufs=1, space="DRAM") as dram_p,
        ):
            dma = nc.sync.dma_start

            # ---- small constants
            biases = const_p.tile([P, KT_U * L], F32, tag="biases_sb", name="biases_sb")
            dma(out=biases[:], in_=bias_d[:])
            scales = const_p.tile([P, NSC], F32, tag="scales_sb", name="scales_sb")
            dma(out=scales[:], in_=scal_d[:])
            dscale = const_p.tile([P, NT], F32, tag="dscale_sb", name="dscale_sb")
            dma(out=dscale[:], in_=dsc_d[:])
            dinv_dst = const_p.tile([P, NT], F32, tag="ddst_sb", name="ddst_sb")
            dma(out=dinv_dst[:], in_=ddst_d[:])
            invdeg = const_p.tile([P, NT], F32, tag="idg_sb", name="idg_sb")
            dma(out=invdeg[:], in_=idg_d[:])
            bo_bc = const_p.tile([P, OUTP], F32, tag="bo_sb", name="bo_sb")
            dma(out=bo_bc[:], in_=bo_d[:])
            ident = const_p.tile([P, P], F32, tag="ident", name="ident")
            make_identity(nc, ident[:])

            # ---- weights resident (bf16)
            w0_sb = []
            for k in range(KT_IN):
                t = w_p.tile([P, UNITS], BF16, tag=f"w0_{k}", name=f"w0_{k}")
                dma(out=t[:], in_=W0_d[k])
                w0_sb.append(t)
            wh_sb = []
            for l in range(L - 1):
                row = []
                for k in range(KT_U):
                    t = w_p.tile([P, UNITS], BF16, tag=f"wh_{l}_{k}", name=f"wh_{l}_{k}")
                    dma(out=t[:], in_=Wh_d[l, k])
                    row.append(t)
                wh_sb.append(row)
            wo_sb = []
            for k in range(KT_U):
                t = w_p.tile([P, OUTP], BF16, tag=f"wo_{k}", name=f"wo_{k}")
                dma(out=t[:], in_=Wo_d[k])
                wo_sb.append(t)

            # ---- A^T resident in SBUF, fp8, pair layout [128, 2, 1280]
            at_sb = []
            for p in range(NPAIR):
                t = at_p.tile([P, 2, NLOC], F8, tag=f"at{p}", name=f"at_{p}")
                dma(out=t[:], in_=ATp_d[p])
                at_sb.append(t)

            # ---- x^T tiles (layer 0 = h^T), bf16
            xT = []
            for k in range(KT_IN):
                t = x_p.tile([P, NLOC], BF16, tag="xT", name=f"xt0_{k}")
                dma(out=t[:], in_=hT_d[k])
                xT.append(t)

            jk = [jk_p.tile([P, NLOC], BF16, tag=f"jk{ft}", name=f"jk_{ft}")
                  for ft in range(KT_U)]

            # ---- collective bounce buffers (per layer, per feat chunk, per part)
            loc_h = [[[dram_p.tile([NT // 2, P, 2, P], mdt,
                                   tag=f"loc{l}_{ch}_{pt}", name=f"loc{l}_{ch}_{pt}")
                       for pt in range(parts)] for ch in range(KT_U)]
                     for l in range(L)]
            full_h = [[[dram_p.tile([NPAIR, P, 2, P], mdt,
                                    tag=f"full{l}_{ch}_{pt}", name=f"full{l}_{ch}_{pt}",
                                    addr_space="Shared")
                        for pt in range(parts)] for ch in range(KT_U)]
                      for l in range(L)]
            loc_f = [dram_p.tile([NT // 2, P, 2, OUTP], mdt, tag=f"locF_{pt}", name=f"locF_{pt}")
                     for pt in range(parts)]
            full_f = [dram_p.tile([NPAIR, P, 2, OUTP], mdt, tag=f"fullF_{pt}", name=f"fullF_{pt}",
                                  addr_space="Shared")
                      for pt in range(parts)]

            def cc(ins_t, outs_t):
                nc.gpsimd.collective_compute(
                    "AllGather", ALU.bypass,
                    replica_groups=[list(range(C))],
                    ins=[ins_t.opt()], outs=[outs_t.opt()],
                )

            def aggregate(ps_slices, full_list, width, lname):
                """DMA gathered pairs, aggregate into per-slice psum tiles.

                Pair-major (stationary m reused across slices); each slice
                accumulates in its OWN psum tile -- sub-bank column-slice
                group interleaving corrupts DR accumulation on HW.
                """
                for p in range(NPAIR):
                    mps = []
                    for pt in range(parts):
                        mp = mr_p.tile([P, 2, width], mdt, tag="mr",
                                       name=f"mr_{lname}_{p}_{pt}")
                        dma(out=mp[:], in_=full_list[pt][p])
                        mps.append(mp)
                    if fp8:
                        for ipt in range(parts):
                            for si, (off, w) in enumerate(slices):
                                nc.tensor.matmul(
                                    ps_slices[si][:, 0:w],
                                    lhsT=mps[ipt][:],
                                    rhs=at_sb[p][:, :, off:off + w],
                                    start=(p == 0 and ipt == 0),
                                    stop=(p == NPAIR - 1 and ipt == parts - 1),
                                    perf_mode=DR,
                                )
                    else:
                        for i in range(2):
                            for si, (off, w) in enumerate(slices):
                                nc.tensor.matmul(
                                    ps_slices[si][:, 0:w],
                                    lhsT=mps[0][:, i, :],
                                    rhs=at_sb[p][:, i, off:off + w],
                                    start=(p == 0 and i == 0),
                                    stop=(p == NPAIR - 1 and i == 1),
                                )

            # ================= hidden layers =================
            for l in range(L):
                w_tiles = w0_sb if l == 0 else wh_sb[l - 1]
                kt = len(xT)
                mloc = ml_p.tile([P, NT, UNITS], mdt, tag="ml",
                                 name=f"ml_{l}")
                mlo = (ml_p.tile([P, NT, UNITS], mdt, tag="mlo",
                                 name=f"mlo_{l}") if parts == 2 else None)
                sc_b = scales[:, 2 * l:2 * l + 1]
                for nt in range(NT):
                    ps = psmm_p.tile([P, UNITS], F32, tag="mm",
                                     name=f"ps_{l}_{nt}")
                    for k in range(kt):
                        nc.tensor.matmul(
                            ps[:], lhsT=xT[k][:, nt * P:(nt + 1) * P],
                            rhs=w_tiles[k][:],
                            start=(k == 0), stop=(k == kt - 1),
                        )
                    # m_hi = fp8(ps * beta_l)
                    nc.scalar.activation(mloc[:, nt, :], ps[:], AF.Identity,
                                         scale=sc_b)
                    if dump:
                        md_t = z_p.tile([P, UNITS], F32, tag="mdmp",
                                        name=f"md_{l}_{nt}")
                        nc.vector.tensor_copy(out=md_t[:], in_=ps[:])
                        dma(out=md_d[l, :, nt, :], in_=md_t[:])
                    if parts == 2:
                        t1 = z_p.tile([P, UNITS], F32, tag="hl1",
                                      name=f"hl1_{l}_{nt}")
                        nc.vector.tensor_scalar(
                            out=t1[:], in0=ps[:], scalar1=sc_b, scalar2=None,
                            op0=ALU.mult)
                        t2 = z_p.tile([P, UNITS], F32, tag="hl2",
                                      name=f"hl2_{l}_{nt}")
                        nc.vector.tensor_copy(out=t2[:], in_=mloc[:, nt, :])
                        nc.vector.tensor_tensor(
                            out=t1[:], in0=t1[:], in1=t2[:], op=ALU.subtract)
                        nc.vector.tensor_copy(out=mlo[:, nt, :], in_=t1[:])
                for q in range(NT // 2):
                    for ch in range(KT_U):
                        dma(out=loc_h[l][ch][0][q],
                            in_=mloc[:, 2 * q:2 * q + 2, ch * P:(ch + 1) * P])
                        if parts == 2:
                            dma(out=loc_h[l][ch][1][q],
                                in_=mlo[:, 2 * q:2 * q + 2, ch * P:(ch + 1) * P])
                for ch in range(KT_U):
                    for pt in range(parts):
                        cc(loc_h[l][ch][pt], full_h[l][ch][pt])

                sc_u = scales[:, 2 * l + 1:2 * l + 2]
                xT = []
                for ft in range(KT_U):
                    pss = [psagg_p.tile([P, SLICE_W], F32, tag=f"agg{si}",
                                        name=f"agg_{l}_{ft}_{si}")
                           for si in range(len(slices))]
                    aggregate(pss, full_h[l][ft], P, f"h{l}_{ft}")
                    xt_new = x_p.tile([P, NLOC], BF16, tag="xT",
                                      name=f"xt{l + 1}_{ft}")
                    for si, (off, w) in enumerate(slices):
                        nc.scalar.activation(
                            xt_new[:, off:off + w], pss[si][:, 0:w], AF.Relu,
                            bias=biases[:, KT_U * l + ft:KT_U * l + ft + 1],
                            scale=sc_u,
                        )
                    xT.append(xt_new)
                    if dump:
                        dma(out=xd_d[l, ft], in_=xt_new[:])
                    if l == 0:
                        nc.vector.tensor_copy(out=jk[ft][:], in_=xt_new[:])
                    else:
                        nc.vector.tensor_tensor(
                            out=jk[ft][:], in0=jk[ft][:], in1=xt_new[:],
                            op=ALU.max)

            # ================= final layer =================
            mo = const_p.tile([P, NT, OUTP], F32, tag="mo_keep", name="mo_keep")
            mloc = ml_p.tile([P, NT, OUTP], mdt, tag="mlf", name="ml_fin")
            mlo = (ml_p.tile([P, NT, OUTP], mdt, tag="mlof", name="mlo_fin")
                   if parts == 2 else None)
            for nt in range(NT):
                psw = psmm_p.tile([P, UNITS], F32, tag="mm", name=f"ps_f_{nt}")
                ps = psw[:, 0:OUTP]
                for k in range(KT_U):
                    nc.tensor.matmul(
                        ps, lhsT=jk[k][:, nt * P:(nt + 1) * P],
                        rhs=wo_sb[k][:],
                        start=(k == 0), stop=(k == KT_U - 1),
                    )
                nc.scalar.activation(mo[:, nt, :], ps, AF.Identity)
                # m_s = fp8(ps * dinv * beta_f)
                nc.vector.tensor_scalar(
                    out=mloc[:, nt, :], in0=ps,
                    scalar1=dscale[:, nt:nt + 1], scalar2=None, op0=ALU.mult)
                if parts == 2:
                    t1 = z_p.tile([P, OUTP], F32, tag="hl1", name=f"hf1_{nt}")
                    nc.vector.tensor_scalar(
                        out=t1[:], in0=ps, scalar1=dscale[:, nt:nt + 1],
                        scalar2=None, op0=ALU.mult)
                    t2 = z_p.tile([P, OUTP], F32, tag="hl2", name=f"hf2_{nt}")
                    nc.vector.tensor_copy(out=t2[:], in_=mloc[:, nt, :])
                    nc.vector.tensor_tensor(out=t1[:], in0=t1[:], in1=t2[:],
                                            op=ALU.subtract)
                    nc.vector.tensor_copy(out=mlo[:, nt, :], in_=t1[:])
            for q in range(NT // 2):
                dma(out=loc_f[0][q], in_=mloc[:, 2 * q:2 * q + 2, :])
                if parts == 2:
                    dma(out=loc_f[1][q], in_=mlo[:, 2 * q:2 * q + 2, :])
            for pt in range(parts):
                cc(loc_f[pt], full_f[pt])

            ps_fin = [psagg_p.tile([P, SLICE_W], F32, tag=f"agg{si}",
                                   name=f"agg_fin_{si}")
                      for si in range(len(slices))]
            aggregate(ps_fin, full_f, OUTP, "fin")

            aggF = af_p.tile([P, NLOC], F32, tag="aggF", name="aggF")
            for si, (off, w) in enumerate(slices):
                nc.scalar.activation(aggF[:, off:off + w], ps_fin[si][:, 0:w],
                                     AF.Identity, scale=scales[:, 13:14])
            for nt in range(NT):
                ps_tw = psmm_p.tile([P, UNITS], F32, tag="mm", name=f"pst_{nt}")
                ps_t = ps_tw[:, 0:P]
                nc.tensor.transpose(
                    out=ps_t, in_=aggF[:, nt * P:(nt + 1) * P],
                    identity=ident[:],
                )
                z = z_p.tile([P, OUT_REAL], F32, tag="z", name=f"z_{nt}")
                nc.vector.tensor_scalar(
                    out=z[:], in0=ps_tw[:, 0:OUT_REAL],
                    scalar1=dinv_dst[:, nt:nt + 1], scalar2=None, op0=ALU.mult)
                t2 = z_p.tile([P, OUT_REAL], F32, tag="zb", name=f"zb_{nt}")
                nc.vector.tensor_scalar(
                    out=t2[:], in0=mo[:, nt, 0:OUT_REAL],
                    scalar1=invdeg[:, nt:nt + 1], scalar2=None, op0=ALU.mult)
                nc.vector.tensor_tensor(out=z[:], in0=z[:], in1=t2[:],
                                        op=ALU.add)
                nc.vector.tensor_tensor(out=z[:], in0=z[:],
                                        in1=bo_bc[:, 0:OUT_REAL], op=ALU.add)
                # log_softmax over the 64 real feats
                rmax = z_p.tile([P, 1], F32, tag="r1", name=f"rmax_{nt}")
                nc.vector.reduce_max(rmax[:], z[:], axis=AX.X)
                nc.vector.tensor_scalar_sub(z[:], z[:], rmax[:])
                ez = z_p.tile([P, OUT_REAL], F32, tag="ez", name=f"ez_{nt}")
                nc.scalar.activation(ez[:], z[:], AF.Exp)
                ssum = z_p.tile([P, 1], F32, tag="r2", name=f"ssum_{nt}")
                nc.vector.reduce_sum(ssum[:], ez[:], axis=AX.X)
                lsum = z_p.tile([P, 1], F32, tag="r3", name=f"lsum_{nt}")
                nc.scalar.activation(lsum[:], ssum[:], AF.Ln)
                o = z_p.tile([P, OUT_REAL], F32, tag="o", name=f"o_{nt}")
                nc.vector.tensor_scalar_sub(o[:], z[:], lsum[:])
                dma(out=out_d[nt * P:(nt + 1) * P, :], in_=o[:])

    nc.compile()
    return nc


# ---------------------------------------------------------------- host prep
def host_prep(h, edge_index, W0, b0, Wh, bh, Wo, bo, mode):
    bf = ml_dtypes.bfloat16
    f8 = ml_dtypes.float8_e4m3
    N = N_NODES
    nlr, nloc = NLOC_REAL, NLOC
    src = np.asarray(edge_index[0], np.int64)
    dst = np.asarray(edge_index[1], np.int64)
    h32 = np.asarray(h, np.float32)
    W0_32 = np.asarray(W0, np.float32)
    Wh_32 = np.asarray(Wh, np.float32)
    Wo_32 = np.asarray(Wo, np.float32)
    b0_32 = np.asarray(b0, np.float32)
    bh_32 = np.asarray(bh, np.float32)
    bo_32 = np.asarray(bo, np.float32)

    deg = np.zeros(N, np.float64)
    np.add.at(deg, dst, 1.0)
    degf = (deg + 1.0).astype(np.float32)
    dinv = degf ** -0.5

    # ---- cheap fp32 forward pass for the fp8 beta_l scales (sort+reduceat
    # segment sum; ~1s).  Only the 7 max|m| scalars are used.
    if mode != "bf16":
        order = np.argsort(dst, kind="stable")
        ds = dst[order]
        ss = src[order]
        seg_starts = np.flatnonzero(np.r_[True, ds[1:] != ds[:-1]])
        seg_ids = ds[seg_starts]

        def seg_sum(m):
            gath = m[ss]
            red = np.add.reduceat(gath, seg_starts, axis=0)
            out = np.zeros((N, m.shape[1]), np.float32)
            out[seg_ids] = red
            return out

        x = h32
        mmax = []
        for l in range(L):
            W = W0_32 if l == 0 else Wh_32[l - 1]
            b = b0_32 if l == 0 else bh_32[l - 1]
            m = x @ W
            mmax.append(float(np.abs(m).max()))
            x = np.maximum(seg_sum(m) + b, 0)
            if l == 0:
                xj = x.copy()
            else:
                np.maximum(xj, x, out=xj)
        mo = xj @ Wo_32
        mfmax = float(np.abs(mo * dinv[:, None]).max())
        betas = [100.0 / max(v, 1e-30) for v in mmax]
        betaf = 100.0 / max(mfmax, 1e-30)
    else:
        betas = [1.0] * L
        betaf = 1.0

    def pad_idx(g):
        return (g // nlr) * nloc + (g % nlr)

    psrc = pad_idx(src)

    scales = np.zeros((P, NSC), np.float32)
    for l in range(L):
        scales[:, 2 * l] = betas[l]
        scales[:, 2 * l + 1] = 1.0 / betas[l]
    scales[:, 13] = 1.0 / betaf

    biases = np.zeros((P, KT_U * L), np.float32)
    for l in range(L):
        b = b0_32 if l == 0 else bh_32[l - 1]
        for ft in range(KT_U):
            biases[:, KT_U * l + ft] = b[ft * P:(ft + 1) * P]
    bo_bc = np.zeros((P, OUTP), np.float32)
    bo_bc[:, :OUT_REAL] = bo_32[None, :]

    Wo_pad = np.zeros((UNITS, OUTP), np.float32)
    Wo_pad[:, :OUT_REAL] = Wo_32

    NFULL = KT * P
    in_maps = []
    for c in range(C):
        lo, hi = c * nlr, min((c + 1) * nlr, N)
        nl = hi - lo
        sel = (dst >= lo) & (dst < hi)
        s_c = psrc[sel]
        d_c = (dst[sel] - lo).astype(np.int64)

        AT = np.zeros((NFULL, nloc), np.float32)
        np.add.at(AT, (s_c, d_c), 1.0)
        ATp = AT.reshape(NPAIR, 2, P, nloc).transpose(0, 2, 1, 3)

        hT = np.zeros((IN_FEATS, nloc), np.float32)
        hT[:, :nl] = h32[lo:hi].T

        gl = np.arange(lo, hi)
        dloc = np.ones(nloc, np.float32)
        dloc[:nl] = dinv[gl]
        dsc = (dloc * betaf).reshape(NT, P).T.copy()
        ddst = dloc.reshape(NT, P).T.copy()
        idg = np.ones(nloc, np.float32)
        idg[:nl] = 1.0 / degf[gl]
        idg = idg.reshape(NT, P).T.copy()

        in_maps.append({
            "hT": hT.reshape(KT_IN, P, nloc).astype(bf).copy(),
            "ATp": ATp.astype(f8).copy(),
            "W0": W0_32.reshape(KT_IN, P, UNITS).astype(bf).copy(),
            "Wh": Wh_32.reshape(L - 1, KT_U, P, UNITS).astype(bf).copy(),
            "Wo": Wo_pad.reshape(KT_U, P, OUTP).astype(bf).copy(),
            "biases": biases,
            "scales": scales,
            "dscale": dsc,
            "dinv_dst": ddst,
            "invdeg": idg,
            "bo_bc": bo_bc,
        })
    return in_maps


_CACHE = {}


def _get_nc():
    if "nc" not in _CACHE:
        _CACHE["nc"] = build_nc(MODE)
    return _CACHE["nc"]


def kernel(h, edge_index, W0, b0, Wh, bh, Wo, bo, _trace=False, _trace_kwargs=None):
    nc = _get_nc()
    in_maps = host_prep(h, edge_index, W0, b0, Wh, bh, Wo, bo, MODE)
    res = run_bass_kernel_spmd(
        nc, in_maps, list(range(C)),
        trace=_trace, **(_trace_kwargs or {}),
    )
    outs = [np.asarray(res.results[c]["out"])[:NLOC_REAL] for c in range(C)]
    full = np.concatenate(outs, axis=0)[:N_NODES].astype(np.float32)
    if _trace:
        return full, res
    return full
